# revision 17
# baseline (speedup 1.0000x reference)
"""Trainium2 Bass kernel for nn_AnalyticalMNet.

Reference computation (per batch b of B=64):
    C       : (2, HW)   concentrations (flattened C_mean)
    Y       : (HW, 3)   optical-density pixels
    gram    = C @ C^T                                  (2,2)
    A       = (1/sigma_sq) I + (1/lambda_sq) gram      (2,2)
    out_var = A^{-1}                                   (2,2)
    yc[s,n] = sum_p Y[p,s] C[n,p]                      (3,2)
    U       = (1/sigma_sq) M_ref + (1/lambda_sq) yc    (3,2)
    out_mean= U @ out_var                              (3,2)

Strategy: pure data parallel over 8 NeuronCores (8 batches per core).
Per batch the heavy work is 9 dot products over HW=262144 elements.
Work split (per batch, tiles are (128, J=2048) views):
  - DVE: two broadcast tensor_tensor multiplies C_n (free-dim step-0
    broadcast) x Y (interleaved, contiguous) -> product planes pr_n
    (128, 6144).  This is the only full-rate elementwise engine, so it
    carries only the 6 yc products (2 passes).
  - PE:  reduces each product plane: ones(128,1) stationary, moving =
    pr_n[:, s::3] in 4 chunks of 512 columns accumulated into a PSUM
    bank (1,512) -> partial sums over partitions+chunks.
  - ACT: fused Square+accum for gram diagonal (C0^2, C1^2) and for
    (C0+C1)^2 (polarization identity for the gram off-diagonal),
    plus the 6 final PSUM (1,512) -> scalar folds per batch.
  - GpSimd: computes C0+C1.
  - gram partials (128,1) are partition-reduced by one ones-matmul.
  - Tiny DVE epilogue: 2x2 inverse + affine update for all batches
    vectorized along the free axis; DMA out (1, 80) per core.
"""

import numpy as np

import concourse.bass as bass
import concourse.tile as tile
from concourse import bacc, mybir
from concourse.bass_utils import run_bass_kernel_spmd

F32 = mybir.dt.float32
BF16 = mybir.dt.bfloat16
AOP = mybir.AluOpType
ACTF = mybir.ActivationFunctionType

B = 64
N_CORES = 8
BPC = B // N_CORES          # batches per core
HW = 512 * 512              # pixels per batch
P = 128                     # SBUF partitions


def build_kernel(bpc=BPC, hw=HW, deint_pr=False):
    """Build the per-core SPMD Bass graph. Returns the Bass object."""
    J = hw // P             # free-dim columns per partition
    W = min(512, J)         # PSUM reduce chunk width
    NCH = (J + W - 1) // W  # chunks per product plane

    nc = bacc.Bacc()
    C_ext = nc.declare_dram_parameter("C", [bpc, 2, hw], F32, isOutput=False)
    Y_ext = nc.declare_dram_parameter("Y", [bpc, hw, 3], F32, isOutput=False)
    M_ext = nc.declare_dram_parameter("M", [1, 6 * bpc], F32, isOutput=False)
    S_ext = nc.declare_dram_parameter("S", [1, 2], F32, isOutput=False)
    O_ext = nc.declare_dram_parameter("out", [1, 10 * bpc], F32, isOutput=True)

    OFFV = 6 * bpc          # out_var offset inside the output row

    with tile.TileContext(nc) as tc:
        with (
            tc.tile_pool(name="const", bufs=1) as const,
            tc.tile_pool(name="data", bufs=3) as data,
            tc.tile_pool(name="prod", bufs=2) as prod,
            tc.tile_pool(name="psum", bufs=2, space="PSUM") as psum,
            tc.tile_pool(name="psg", bufs=1, space="PSUM") as psg,
        ):
            ones = const.tile([P, 1], F32)
            nc.vector.memset(ones[:], 1.0)
            ones_bf = const.tile([P, 1], BF16)
            nc.vector.memset(ones_bf[:], 1.0)
            # gram partials: cols 3b+{0,1,2} = G00, G01, G11
            accums = const.tile([P, 3 * bpc], F32)
            # yc sums, written by ACT folds: col 6b + 2s + n
            sums_yc = const.tile([1, 6 * bpc], F32)
            m_t = const.tile([1, 6 * bpc], F32)
            nc.sync.dma_start(m_t[:], M_ext[:])
            sl_t = const.tile([1, 2], F32)
            nc.sync.dma_start(sl_t[:], S_ext[:])

            for b in range(bpc):
                c_t = data.tile([P, 2, J], F32, tag="c")
                nc.sync.dma_start(
                    c_t[:], C_ext[b].rearrange("n (p j) -> p n j", p=P)
                )
                y_t = data.tile([P, J, 3], F32, tag="y")
                y_src = Y_ext[b].rearrange("(p j) s -> p j s", p=P)
                if b == 0 and J >= 2:
                    # split the first Y transfer so compute starts earlier
                    nc.sync.dma_start(y_t[:, : J // 2, :], y_src[:, : J // 2, :])
                    nc.sync.dma_start(y_t[:, J // 2 :, :], y_src[:, J // 2 :, :])
                else:
                    nc.sync.dma_start(y_t[:], y_src)
                # gram: diagonal squares on ACT, off-diagonal fused on DVE
                sq = prod.tile([P, J], BF16, tag="sq")
                nc.scalar.activation(
                    sq[:], c_t[:, 0, :], ACTF.Square,
                    accum_out=accums[:, 3 * b : 3 * b + 1],
                )
                sq = prod.tile([P, J], BF16, tag="sq")
                nc.scalar.activation(
                    sq[:], c_t[:, 1, :], ACTF.Square,
                    accum_out=accums[:, 3 * b + 2 : 3 * b + 3],
                )
                g01sc = prod.tile([P, J], F32, tag="g01sc")
                nc.vector.scalar_tensor_tensor(
                    g01sc[:], c_t[:, 0, :], 1.0, c_t[:, 1, :],
                    op0=AOP.mult, op1=AOP.mult,
                    accum_out=accums[:, 3 * b + 1 : 3 * b + 2],
                )
                # yc products: broadcast-TT then phase-aligned PE reduce.
                # Moving chunks are CONTIGUOUS runs of the interleaved
                # product; W3 % 3 == 0 keeps psum col n <-> s = n mod 3.
                halves = [(0, J)]
                if b == 0 and J >= 8:
                    halves = [(0, J // 2), (J // 2, J)]
                FLH = 3 * min(j1 - j0 for j0, j1 in halves)
                W3 = min(510, FLH)
                ps_a = psum.tile([1, W3], F32, tag="psa")
                ps_b = psum.tile([1, W3], F32, tag="psb")
                ps_n = [ps_a, ps_b]
                first = [True, True]
                for hi, (j0, j1) in enumerate(halves):
                    # one merged TT covers both n planes of this j-range
                    cn = c_t[:, :, j0:j1]
                    cb = bass.AP(
                        cn.tensor, cn.offset,
                        [list(d) for d in cn.ap] + [[0, 3]],
                    )
                    yv = y_t[:, j0:j1, :]
                    yb = bass.AP(
                        yv.tensor, yv.offset,
                        [list(yv.ap[0]), [0, 2]] + [list(d) for d in yv.ap[1:]],
                    )
                    pr = prod.tile([P, 2, j1 - j0, 3], BF16, tag="pr")
                    nc.vector.tensor_mul(pr[:], cb, yb)
                    prflat = pr[:].rearrange("p n j s -> p n (j s)")
                    FL0 = 3 * (j1 - j0)
                    for n in range(2):
                        ofs = 0
                        while ofs < FL0:
                            w = min(W3, FL0 - ofs)
                            nc.tensor.matmul(
                                ps_n[n][:, 0:w],
                                ones_bf[:],
                                prflat[:, n, ofs : ofs + w],
                                start=first[n],
                                stop=(hi == len(halves) - 1 and ofs + w >= FL0),
                            )
                            first[n] = False
                            ofs += W3
                # fold psum rows (stride-3 views) into the per-batch yc sums
                for n in range(2):
                    psv = ps_n[n][:].rearrange("p (j s) -> p s j", s=3)
                    for s in range(3):
                        col = 6 * b + 2 * s + n
                        fsc = prod.tile([1, W3 // 3], BF16, tag="fsc")
                        nc.scalar.activation(
                            fsc[:], psv[:, s, :], ACTF.Copy,
                            accum_out=sums_yc[:, col : col + 1],
                        )

            # cross-partition reduction of gram partials -> (1, 3*bpc)
            sums_gp = psg.tile([1, 3 * bpc], F32)
            nc.tensor.matmul(sums_gp[:], ones[:], accums[:], start=True, stop=True)
            sg = const.tile([1, 3 * bpc], F32)
            nc.vector.tensor_copy(sg[:], sums_gp[:])

            # ---- epilogue: 2x2 inverse + affine update, vectorized over b ----
            out_t = const.tile([1, 10 * bpc], F32)
            recips = const.tile([1, 2], F32)
            nc.vector.reciprocal(recips[:], sl_t[:])
            isg = recips[:, 0:1]       # 1/sigma_sq
            ilq = recips[:, 1:2]       # 1/lambda_sq
            sgv = sg[:].rearrange("p (b k) -> p b k", b=bpc)
            g00 = sgv[:, :, 0]
            g01 = sgv[:, :, 1]
            g11 = sgv[:, :, 2]

            # A = isg*I + ilq*gram
            e_a = const.tile([1, bpc], F32)
            nc.vector.tensor_scalar(e_a[:], g00, ilq, isg, op0=AOP.mult, op1=AOP.add)
            e_d = const.tile([1, bpc], F32)
            nc.vector.tensor_scalar(e_d[:], g11, ilq, isg, op0=AOP.mult, op1=AOP.add)
            e_b = const.tile([1, bpc], F32)
            nc.vector.tensor_scalar_mul(e_b[:], g01, ilq)

            t_ad = const.tile([1, bpc], F32)
            nc.vector.tensor_mul(t_ad[:], e_a[:], e_d[:])
            t_b2 = const.tile([1, bpc], F32)
            nc.vector.tensor_mul(t_b2[:], e_b[:], e_b[:])
            det = const.tile([1, bpc], F32)
            nc.vector.tensor_sub(det[:], t_ad[:], t_b2[:])
            rdet = const.tile([1, bpc], F32)
            nc.vector.reciprocal(rdet[:], det[:])

            # out_var = adj(A) * rdet, stored [v00 v01 v01 v11] per batch
            vv = out_t[:, OFFV:].rearrange("p (b k) -> p b k", k=4)
            nc.vector.tensor_mul(vv[:, :, 0], e_d[:], rdet[:])
            nc.vector.scalar_tensor_tensor(
                vv[:, :, 1], e_b[:], -1.0, rdet[:], op0=AOP.mult, op1=AOP.mult
            )
            nc.vector.tensor_copy(vv[:, :, 2], vv[:, :, 1])
            nc.vector.tensor_mul(vv[:, :, 3], e_a[:], rdet[:])

            # U = isg*M_ref + ilq*yc   (both laid out (b, s, n))
            w_yc = const.tile([1, 6 * bpc], F32)
            nc.vector.tensor_scalar_mul(w_yc[:], sums_yc[:], ilq)
            u_t = const.tile([1, 6 * bpc], F32)
            nc.vector.scalar_tensor_tensor(
                u_t[:], m_t[:], isg, w_yc[:], op0=AOP.mult, op1=AOP.add
            )

            # out_mean[b,s,n] = U[b,s,0]*V[b,0,n] + U[b,s,1]*V[b,1,n]
            u4 = u_t[:].rearrange("p (b s n) -> p b s n", b=bpc, s=3)
            t1 = const.tile([1, 6 * bpc], F32)
            t1v = t1[:].rearrange("p (b s n) -> p b s n", b=bpc, s=3)
            t2 = const.tile([1, 6 * bpc], F32)
            t2v = t2[:].rearrange("p (b s n) -> p b s n", b=bpc, s=3)
            shp = (1, bpc, 3, 2)

            def _bcast_mid(ap_in, n):
                # insert a stride-0 dim of size n before the last dim
                dims = [list(d) for d in ap_in.ap]
                dims.insert(len(dims) - 1, [0, n])
                return bass.AP(ap_in.tensor, ap_in.offset, dims)

            nc.vector.tensor_mul(
                t1v[:, :, :, :],
                u4[:, :, :, 0:1].broadcast_to(shp),
                _bcast_mid(vv[:, :, 0:2], 3),
            )
            nc.vector.tensor_mul(
                t2v[:, :, :, :],
                u4[:, :, :, 1:2].broadcast_to(shp),
                _bcast_mid(vv[:, :, 1:4:2], 3),
            )
            nc.vector.tensor_add(out_t[:, 0:OFFV], t1[:], t2[:])

            nc.sync.dma_start(O_ext[:], out_t[:])

    nc.compile()
    return nc


def _postprocess(rows, bpc):
    """rows: list of per-core (1, 10*bpc) results -> (out_mean, out_var)."""
    means, varis = [], []
    for r in rows:
        r = np.asarray(r).reshape(-1)
        means.append(r[: 6 * bpc].reshape(bpc, 3, 2))
        varis.append(r[6 * bpc :].reshape(bpc, 2, 2))
    return np.concatenate(means, 0), np.concatenate(varis, 0)


def make_in_maps(Y_OD, C_mean, M_ref, sigma_sq, lambda_sq, bpc=BPC, n_cores=N_CORES):
    Y_OD = np.ascontiguousarray(np.asarray(Y_OD, dtype=np.float32))
    C_mean = np.ascontiguousarray(np.asarray(C_mean, dtype=np.float32))
    M_ref = np.ascontiguousarray(np.asarray(M_ref, dtype=np.float32))
    hw = Y_OD.shape[1]
    sl = np.array([[np.float32(sigma_sq), np.float32(lambda_sq)]], dtype=np.float32)
    in_maps = []
    for i in range(n_cores):
        lo, hi = i * bpc, (i + 1) * bpc
        in_maps.append(
            {
                "C": C_mean[lo:hi].reshape(bpc, 2, hw),
                "Y": Y_OD[lo:hi],
                "M": M_ref[lo:hi].reshape(1, 6 * bpc),
                "S": sl,
            }
        )
    return in_maps


_NC_CACHE = {}


import os


def _get_nc(bpc, hw):
    deint = os.environ.get("K_DEINT", "0") == "1"
    key = (bpc, hw, deint)
    if key not in _NC_CACHE:
        _NC_CACHE[key] = build_kernel(bpc, hw, deint_pr=deint)
    return _NC_CACHE[key]


def kernel(Y_OD, C_mean, M_ref, sigma_sq, lambda_sq, trace=False, **trace_kwargs):
    nc = _get_nc(BPC, HW)
    in_maps = make_in_maps(Y_OD, C_mean, M_ref, sigma_sq, lambda_sq)
    res = run_bass_kernel_spmd(
        nc, in_maps, core_ids=list(range(N_CORES)), trace=trace, **trace_kwargs
    )
    out_mean, out_var = _postprocess([m["out"] for m in res.results], BPC)
    if trace:
        kernel.last_exec_time_ns = res.exec_time_ns
        kernel.last_results = res
    return out_mean, out_var


# revision 18
# speedup vs baseline: 1.0640x; 1.0640x over previous
"""Trainium2 Bass kernel for nn_AnalyticalMNet.

Reference computation (per batch b of B=64):
    C       : (2, HW)   concentrations (flattened C_mean)
    Y       : (HW, 3)   optical-density pixels
    gram    = C @ C^T                                  (2,2)
    A       = (1/sigma_sq) I + (1/lambda_sq) gram      (2,2)
    out_var = A^{-1}                                   (2,2)
    yc[s,n] = sum_p Y[p,s] C[n,p]                      (3,2)
    U       = (1/sigma_sq) M_ref + (1/lambda_sq) yc    (3,2)
    out_mean= U @ out_var                              (3,2)

Strategy: pure data parallel over 8 NeuronCores (8 batches per core).
Per batch the heavy work is 9 dot products over HW=262144 elements.
Work split (per batch, tiles are (128, J=2048) views):
  - DVE: two broadcast tensor_tensor multiplies C_n (free-dim step-0
    broadcast) x Y (interleaved, contiguous) -> product planes pr_n
    (128, 6144).  This is the only full-rate elementwise engine, so it
    carries only the 6 yc products (2 passes).
  - PE:  reduces each product plane: ones(128,1) stationary, moving =
    pr_n[:, s::3] in 4 chunks of 512 columns accumulated into a PSUM
    bank (1,512) -> partial sums over partitions+chunks.
  - ACT: fused Square+accum for gram diagonal (C0^2, C1^2) and for
    (C0+C1)^2 (polarization identity for the gram off-diagonal),
    plus the 6 final PSUM (1,512) -> scalar folds per batch.
  - GpSimd: computes C0+C1.
  - gram partials (128,1) are partition-reduced by one ones-matmul.
  - Tiny DVE epilogue: 2x2 inverse + affine update for all batches
    vectorized along the free axis; DMA out (1, 80) per core.
"""

import numpy as np

import concourse.bass as bass
import concourse.tile as tile
from concourse import bacc, mybir
from concourse.bass_utils import run_bass_kernel_spmd

F32 = mybir.dt.float32
BF16 = mybir.dt.bfloat16
AOP = mybir.AluOpType
ACTF = mybir.ActivationFunctionType

B = 64
N_CORES = 8
BPC = B // N_CORES          # batches per core
HW = 512 * 512              # pixels per batch
P = 128                     # SBUF partitions


def build_kernel(bpc=BPC, hw=HW, deint_pr=False):
    """Build the per-core SPMD Bass graph. Returns the Bass object."""
    J = hw // P             # free-dim columns per partition
    W = min(512, J)         # PSUM reduce chunk width
    NCH = (J + W - 1) // W  # chunks per product plane

    nc = bacc.Bacc()
    C_ext = nc.declare_dram_parameter("C", [bpc, 2, hw], F32, isOutput=False)
    Y_ext = nc.declare_dram_parameter("Y", [bpc, hw, 3], F32, isOutput=False)
    M_ext = nc.declare_dram_parameter("M", [1, 6 * bpc], F32, isOutput=False)
    S_ext = nc.declare_dram_parameter("S", [1, 2], F32, isOutput=False)
    O_ext = nc.declare_dram_parameter("out", [1, 10 * bpc], F32, isOutput=True)

    OFFV = 6 * bpc          # out_var offset inside the output row

    with tile.TileContext(nc) as tc:
        with (
            tc.tile_pool(name="const", bufs=1) as const,
            tc.tile_pool(name="data", bufs=3) as data,
            tc.tile_pool(name="prod", bufs=2) as prod,
            tc.tile_pool(name="psum", bufs=2, space="PSUM") as psum,
            tc.tile_pool(name="psg", bufs=1, space="PSUM") as psg,
        ):
            ones = const.tile([P, 1], F32)
            nc.vector.memset(ones[:], 1.0)
            ones_bf = const.tile([P, 1], BF16)
            nc.vector.memset(ones_bf[:], 1.0)
            # gram partials: cols 3b+{0,1,2} = G00, G01, G11
            accums = const.tile([P, 3 * bpc], F32)
            # yc sums, written by ACT folds: col 6b + 2s + n
            sums_yc = const.tile([1, 6 * bpc], F32)
            m_t = const.tile([1, 6 * bpc], F32)
            nc.sync.dma_start(m_t[:], M_ext[:])
            sl_t = const.tile([1, 2], F32)
            nc.sync.dma_start(sl_t[:], S_ext[:])

            for b in range(bpc):
                c_t = data.tile([P, 2, J], F32, tag="c")
                nc.sync.dma_start(
                    c_t[:], C_ext[b].rearrange("n (p j) -> p n j", p=P)
                )
                y_t = data.tile([P, J, 3], F32, tag="y")
                y_src = Y_ext[b].rearrange("(p j) s -> p j s", p=P)
                if b == 0 and J >= 2:
                    # split the first Y transfer so compute starts earlier
                    nc.sync.dma_start(y_t[:, : J // 2, :], y_src[:, : J // 2, :])
                    nc.sync.dma_start(y_t[:, J // 2 :, :], y_src[:, J // 2 :, :])
                else:
                    nc.sync.dma_start(y_t[:], y_src)
                # gram: diagonal squares on ACT, off-diagonal fused on DVE
                sq = prod.tile([P, J], BF16, tag="sq")
                nc.scalar.activation(
                    sq[:], c_t[:, 0, :], ACTF.Square,
                    accum_out=accums[:, 3 * b : 3 * b + 1],
                )
                sq = prod.tile([P, J], BF16, tag="sq")
                nc.scalar.activation(
                    sq[:], c_t[:, 1, :], ACTF.Square,
                    accum_out=accums[:, 3 * b + 2 : 3 * b + 3],
                )
                g01sc = prod.tile([P, J], F32, tag="g01sc")
                nc.vector.scalar_tensor_tensor(
                    g01sc[:], c_t[:, 0, :], 1.0, c_t[:, 1, :],
                    op0=AOP.mult, op1=AOP.mult,
                    accum_out=accums[:, 3 * b + 1 : 3 * b + 2],
                )
                # yc products: broadcast-TT then phase-aligned PE reduce.
                # Moving chunks are CONTIGUOUS runs of the interleaved
                # product; W3 % 3 == 0 keeps psum col n <-> s = n mod 3.
                halves = [(0, J)]
                if b == 0 and J >= 8:
                    halves = [(0, J // 2), (J // 2, J)]
                FLH = 3 * min(j1 - j0 for j0, j1 in halves)
                W3 = min(510, FLH)
                ps_a = psum.tile([1, W3], F32, tag="psa")
                ps_b = psum.tile([1, W3], F32, tag="psb")
                ps_n = [ps_a, ps_b]
                for n in range(2):
                    first = True
                    for hi, (j0, j1) in enumerate(halves):
                        cn = c_t[:, n, j0:j1]
                        cb = bass.AP(
                            cn.tensor, cn.offset,
                            [list(d) for d in cn.ap] + [[0, 3]],
                        )
                        pr = prod.tile([P, j1 - j0, 3], BF16, tag=f"pr{n}")
                        nc.vector.tensor_mul(pr[:], cb, y_t[:, j0:j1, :])
                        prflat = pr[:].rearrange("p j s -> p (j s)")
                        FL0 = 3 * (j1 - j0)
                        ofs = 0
                        while ofs < FL0:
                            w = min(W3, FL0 - ofs)
                            nc.tensor.matmul(
                                ps_n[n][:, 0:w],
                                ones_bf[:],
                                prflat[:, ofs : ofs + w],
                                start=first,
                                stop=(hi == len(halves) - 1 and ofs + w >= FL0),
                            )
                            first = False
                            ofs += W3
                # fold psum rows (stride-3 views) into the per-batch yc sums
                for n in range(2):
                    psv = ps_n[n][:].rearrange("p (j s) -> p s j", s=3)
                    for s in range(3):
                        col = 6 * b + 2 * s + n
                        fsc = prod.tile([1, W3 // 3], BF16, tag="fsc")
                        nc.scalar.activation(
                            fsc[:], psv[:, s, :], ACTF.Copy,
                            accum_out=sums_yc[:, col : col + 1],
                        )

            # cross-partition reduction of gram partials -> (1, 3*bpc)
            sums_gp = psg.tile([1, 3 * bpc], F32)
            nc.tensor.matmul(sums_gp[:], ones[:], accums[:], start=True, stop=True)
            sg = const.tile([1, 3 * bpc], F32)
            nc.vector.tensor_copy(sg[:], sums_gp[:])

            # ---- epilogue: 2x2 inverse + affine update, vectorized over b ----
            out_t = const.tile([1, 10 * bpc], F32)
            recips = const.tile([1, 2], F32)
            nc.vector.reciprocal(recips[:], sl_t[:])
            isg = recips[:, 0:1]       # 1/sigma_sq
            ilq = recips[:, 1:2]       # 1/lambda_sq
            sgv = sg[:].rearrange("p (b k) -> p b k", b=bpc)
            g00 = sgv[:, :, 0]
            g01 = sgv[:, :, 1]
            g11 = sgv[:, :, 2]

            # A = isg*I + ilq*gram
            e_a = const.tile([1, bpc], F32)
            nc.vector.tensor_scalar(e_a[:], g00, ilq, isg, op0=AOP.mult, op1=AOP.add)
            e_d = const.tile([1, bpc], F32)
            nc.vector.tensor_scalar(e_d[:], g11, ilq, isg, op0=AOP.mult, op1=AOP.add)
            e_b = const.tile([1, bpc], F32)
            nc.vector.tensor_scalar_mul(e_b[:], g01, ilq)

            t_ad = const.tile([1, bpc], F32)
            nc.vector.tensor_mul(t_ad[:], e_a[:], e_d[:])
            t_b2 = const.tile([1, bpc], F32)
            nc.vector.tensor_mul(t_b2[:], e_b[:], e_b[:])
            det = const.tile([1, bpc], F32)
            nc.vector.tensor_sub(det[:], t_ad[:], t_b2[:])
            rdet = const.tile([1, bpc], F32)
            nc.vector.reciprocal(rdet[:], det[:])

            # out_var = adj(A) * rdet, stored [v00 v01 v01 v11] per batch
            vv = out_t[:, OFFV:].rearrange("p (b k) -> p b k", k=4)
            nc.vector.tensor_mul(vv[:, :, 0], e_d[:], rdet[:])
            nc.vector.scalar_tensor_tensor(
                vv[:, :, 1], e_b[:], -1.0, rdet[:], op0=AOP.mult, op1=AOP.mult
            )
            nc.vector.tensor_copy(vv[:, :, 2], vv[:, :, 1])
            nc.vector.tensor_mul(vv[:, :, 3], e_a[:], rdet[:])

            # U = isg*M_ref + ilq*yc   (both laid out (b, s, n))
            w_yc = const.tile([1, 6 * bpc], F32)
            nc.vector.tensor_scalar_mul(w_yc[:], sums_yc[:], ilq)
            u_t = const.tile([1, 6 * bpc], F32)
            nc.vector.scalar_tensor_tensor(
                u_t[:], m_t[:], isg, w_yc[:], op0=AOP.mult, op1=AOP.add
            )

            # out_mean[b,s,n] = U[b,s,0]*V[b,0,n] + U[b,s,1]*V[b,1,n]
            u4 = u_t[:].rearrange("p (b s n) -> p b s n", b=bpc, s=3)
            t1 = const.tile([1, 6 * bpc], F32)
            t1v = t1[:].rearrange("p (b s n) -> p b s n", b=bpc, s=3)
            t2 = const.tile([1, 6 * bpc], F32)
            t2v = t2[:].rearrange("p (b s n) -> p b s n", b=bpc, s=3)
            shp = (1, bpc, 3, 2)

            def _bcast_mid(ap_in, n):
                # insert a stride-0 dim of size n before the last dim
                dims = [list(d) for d in ap_in.ap]
                dims.insert(len(dims) - 1, [0, n])
                return bass.AP(ap_in.tensor, ap_in.offset, dims)

            nc.vector.tensor_mul(
                t1v[:, :, :, :],
                u4[:, :, :, 0:1].broadcast_to(shp),
                _bcast_mid(vv[:, :, 0:2], 3),
            )
            nc.vector.tensor_mul(
                t2v[:, :, :, :],
                u4[:, :, :, 1:2].broadcast_to(shp),
                _bcast_mid(vv[:, :, 1:4:2], 3),
            )
            nc.vector.tensor_add(out_t[:, 0:OFFV], t1[:], t2[:])

            nc.sync.dma_start(O_ext[:], out_t[:])

    nc.compile()
    return nc


def _postprocess(rows, bpc):
    """rows: list of per-core (1, 10*bpc) results -> (out_mean, out_var)."""
    means, varis = [], []
    for r in rows:
        r = np.asarray(r).reshape(-1)
        means.append(r[: 6 * bpc].reshape(bpc, 3, 2))
        varis.append(r[6 * bpc :].reshape(bpc, 2, 2))
    return np.concatenate(means, 0), np.concatenate(varis, 0)


def make_in_maps(Y_OD, C_mean, M_ref, sigma_sq, lambda_sq, bpc=BPC, n_cores=N_CORES):
    Y_OD = np.ascontiguousarray(np.asarray(Y_OD, dtype=np.float32))
    C_mean = np.ascontiguousarray(np.asarray(C_mean, dtype=np.float32))
    M_ref = np.ascontiguousarray(np.asarray(M_ref, dtype=np.float32))
    hw = Y_OD.shape[1]
    sl = np.array([[np.float32(sigma_sq), np.float32(lambda_sq)]], dtype=np.float32)
    in_maps = []
    for i in range(n_cores):
        lo, hi = i * bpc, (i + 1) * bpc
        in_maps.append(
            {
                "C": C_mean[lo:hi].reshape(bpc, 2, hw),
                "Y": Y_OD[lo:hi],
                "M": M_ref[lo:hi].reshape(1, 6 * bpc),
                "S": sl,
            }
        )
    return in_maps


_NC_CACHE = {}


import os


def _get_nc(bpc, hw):
    deint = os.environ.get("K_DEINT", "0") == "1"
    key = (bpc, hw, deint)
    if key not in _NC_CACHE:
        _NC_CACHE[key] = build_kernel(bpc, hw, deint_pr=deint)
    return _NC_CACHE[key]


def kernel(Y_OD, C_mean, M_ref, sigma_sq, lambda_sq, trace=False, **trace_kwargs):
    nc = _get_nc(BPC, HW)
    in_maps = make_in_maps(Y_OD, C_mean, M_ref, sigma_sq, lambda_sq)
    res = run_bass_kernel_spmd(
        nc, in_maps, core_ids=list(range(N_CORES)), trace=trace, **trace_kwargs
    )
    out_mean, out_var = _postprocess([m["out"] for m in res.results], BPC)
    if trace:
        kernel.last_exec_time_ns = res.exec_time_ns
        kernel.last_results = res
    return out_mean, out_var
